# revision 1
# baseline (speedup 1.0000x reference)
"""Sparse 3D conv (rulebook gather -> GEMM -> accumulate) on 8 TRN2 NeuronCores.

Strategy (data-parallel over output sites, no collectives):
  - Replicate the feats table (bf16, with a trailing zero row for invalid
    rulebook entries) and the [27,64,64] kernel on every core.
  - Shard the 400k output sites 50k/core; each core gathers its neighbor
    rows via SWDGE indirect DMA (one big gather per 512-site tile covering
    all 27 kernel offsets), transposes gathered [site, cin] tiles to
    [cin, site] on the TensorEngine (pairs of offsets packed to K=128),
    and accumulates 14 K=128-packed bf16 matmuls into f32 PSUM.
  - Bias add fused into the PSUM->SBUF copy, output transposed back to
    site-major on the TensorEngine and DMAd out contiguously.
"""

import os
import sys
from contextlib import ExitStack

sys.path.insert(0, "/opt/trn_rl_repo")

import ml_dtypes
import numpy as np

import concourse.bass as bass
import concourse.tile as tile
from concourse import bacc, mybir
from concourse.bass_utils import run_bass_kernel_spmd
from concourse.masks import make_identity

BF16 = ml_dtypes.bfloat16

# Problem constants (hardcoded per contract)
N = 400000
CIN = 64
COUT = 64
KVOL = 27
NCORES = 8
SPC = N // NCORES  # sites per core = 50000

TILE = 512  # sites per device tile
NPAIRS = (KVOL + 1) // 2  # 14 (27 offsets + 1 zero pad)
IDX_PER_TILE = NPAIRS * 8  # 112 indices per partition per tile


class Cfg:
    def __init__(self, n_rows, n_tiles):
        self.n_rows = n_rows  # feats table rows incl. zero pad rows
        self.n_tiles = n_tiles  # site tiles per core
        self.spad = n_tiles * TILE  # padded sites per core


N_SWDGE_QUEUES = 1  # spread indirect gathers across SWDGE queues when >1
GCOLS = 1  # index columns per indirect-DMA instruction (HW: 1 desc/partition)


def build(cfg: Cfg):
    """Build + compile the per-core Bass program. Returns (nc, names)."""
    nc = bacc.Bacc(
        "TRN2",
        target_bir_lowering=False,
        debug=False,
        num_devices=NCORES,
        num_swdge_queues=N_SWDGE_QUEUES,
    )
    f32 = mybir.dt.float32
    bf16 = mybir.dt.bfloat16
    i32 = mybir.dt.int32

    tbl = nc.dram_tensor("tbl", [cfg.n_rows, CIN], bf16, kind="ExternalInput")
    cent = nc.dram_tensor("cent", [cfg.spad, CIN], bf16, kind="ExternalInput")
    idxs = nc.dram_tensor(
        "idxs", [cfg.n_tiles, 128, IDX_PER_TILE], i32, kind="ExternalInput"
    )
    wts = nc.dram_tensor("wts", [128, NPAIRS * COUT], bf16, kind="ExternalInput")
    biasd = nc.dram_tensor("bias", [COUT, 1], f32, kind="ExternalInput")
    outd = nc.dram_tensor("out", [cfg.spad, COUT], f32, kind="ExternalOutput")

    with tile.TileContext(nc) as tc, ExitStack() as ctx:
        const = ctx.enter_context(tc.tile_pool(name="const", bufs=1))
        idf = const.tile([128, 128], f32)
        make_identity(nc, idf[:])
        idb = const.tile([128, 128], bf16)
        nc.vector.tensor_copy(idb[:], idf[:])
        wt = const.tile([128, NPAIRS * COUT], bf16)
        nc.sync.dma_start(wt[:], wts[:])
        bt = const.tile([COUT, 1], f32)
        nc.sync.dma_start(bt[:], biasd[:])

        ip = ctx.enter_context(tc.tile_pool(name="ip", bufs=3))
        gp = ctx.enter_context(tc.tile_pool(name="gp", bufs=3))
        tp = ctx.enter_context(tc.tile_pool(name="tp", bufs=2, space="PSUM"))
        rp = ctx.enter_context(tc.tile_pool(name="rp", bufs=6))
        app = ctx.enter_context(tc.tile_pool(name="ap", bufs=2, space="PSUM"))
        bp = ctx.enter_context(tc.tile_pool(name="bp", bufs=2))
        otp = ctx.enter_context(tc.tile_pool(name="otp", bufs=2, space="PSUM"))
        osp = ctx.enter_context(tc.tile_pool(name="osp", bufs=3))

        gq = [0]

        def gather(out_ap, idx_ap):
            inst = nc.gpsimd.indirect_dma_start(
                out=out_ap,
                out_offset=None,
                in_=tbl[:],
                in_offset=bass.IndirectOffsetOnAxis(ap=idx_ap, axis=0),
            )
            if N_SWDGE_QUEUES > 1:
                q = gq[0] % N_SWDGE_QUEUES
                gq[0] += 1
                if q:
                    inst.ins.queue = f"qPoolDynamic{q}"
            return inst

        for t in range(cfg.n_tiles):
            it = ip.tile([128, IDX_PER_TILE], i32)
            nc.sync.dma_start(it[:], idxs[t])
            acc = app.tile([COUT, TILE], f32)
            # Batched indirect gather: each instruction carries GCOLS index
            # columns (128*GCOLS descriptors); out column block q*64 holds
            # tbl[it[p, q]] for index column q = pr*8 + c*2 + w, so the
            # (pr, c) transpose block sits at cols pr*512 + c*128.
            g = gp.tile([128, IDX_PER_TILE * CIN], bf16)
            # center offset k=13 -> pr=6, w=1 -> index columns 49,51,53,55:
            # rows are the tile's own contiguous sites; feed them via plain
            # HWDGE DMA instead of Pool-engine indirect gathers.
            CQ = (49, 51, 53, 55)
            for gi in range(IDX_PER_TILE // GCOLS):
                if GCOLS == 1 and gi in CQ:
                    continue
                gather(
                    g[:, gi * GCOLS * CIN : (gi + 1) * GCOLS * CIN],
                    it[:, gi * GCOLS : (gi + 1) * GCOLS],
                )
            if GCOLS == 1:
                for c in range(4):
                    q = CQ[c]
                    nc.sync.dma_start(
                        g[:, q * CIN : (q + 1) * CIN],
                        cent[t * TILE + c * 128 : t * TILE + (c + 1) * 128],
                    )
            for pr in range(NPAIRS):
                tpt = tp.tile([128, TILE], bf16)
                for c in range(4):
                    nc.tensor.transpose(
                        out=tpt[:, c * 128 : (c + 1) * 128],
                        in_=g[:, pr * 512 + c * 128 : pr * 512 + (c + 1) * 128],
                        identity=idb[:],
                    )
                r = rp.tile([128, TILE], bf16)
                if pr % 2 == 0:
                    nc.vector.tensor_copy(r[:], tpt[:])
                else:
                    nc.scalar.copy(r[:], tpt[:])
                nc.tensor.matmul(
                    acc[:],
                    wt[:, pr * COUT : (pr + 1) * COUT],
                    r[:],
                    start=(pr == 0),
                    stop=(pr == NPAIRS - 1),
                )
            ob = bp.tile([COUT, TILE], f32)
            nc.vector.tensor_add(
                out=ob[:], in0=acc[:], in1=bt[:].to_broadcast([COUT, TILE])
            )
            ot = otp.tile([128, 4 * COUT], f32)
            for c in range(4):
                nc.tensor.transpose(
                    out=ot[:, c * COUT : (c + 1) * COUT],
                    in_=ob[:, c * 128 : (c + 1) * 128],
                    identity=idf[:COUT, :COUT],
                )
            os_ = osp.tile([128, 4 * COUT], f32)
            nc.scalar.copy(os_[:], ot[:])
            nc.sync.dma_start(
                outd[t * TILE : (t + 1) * TILE, :].rearrange(
                    "(c p) ci -> p c ci", p=128
                ),
                os_[:].rearrange("p (c ci) -> p c ci", c=4),
            )

    nc.compile()
    return nc


def prep_inputs(feats, kern, bias, neighbor_map, cfg: Cfg, n_sites_total, n_cores):
    """Host-side marshalling into per-core input maps."""
    zrow = n_sites_total  # index of the zero row in the padded table
    assert cfg.n_rows > zrow

    tblh = np.zeros((cfg.n_rows, CIN), dtype=BF16)
    tblh[: feats.shape[0]] = feats.astype(BF16)

    nm = np.asarray(neighbor_map)
    idx32 = np.where(nm >= 0, nm, zrow).astype(np.int32)  # [KVOL, n_sites]

    w_pk = np.zeros((NPAIRS, 128, COUT), dtype=np.float32)
    for pr in range(NPAIRS):
        k0, k1 = 2 * pr, 2 * pr + 1
        w_pk[pr, :CIN] = kern[k0]
        if k1 < KVOL:
            w_pk[pr, CIN:] = kern[k1]
    wtsh = np.ascontiguousarray(
        w_pk.transpose(1, 0, 2).reshape(128, NPAIRS * COUT)
    ).astype(BF16)

    biash = np.ascontiguousarray(bias.reshape(COUT, 1)).astype(np.float32)

    spc = n_sites_total // n_cores
    in_maps = []
    for c in range(n_cores):
        sl = idx32[:, c * spc : (c + 1) * spc]  # [27, spc]
        padn = cfg.spad - spc
        a = np.concatenate(
            [
                np.concatenate(
                    [sl, np.full((KVOL, padn), zrow, np.int32)], axis=1
                ),
                np.full((1, cfg.spad), zrow, np.int32),
            ],
            axis=0,
        )  # [28, spad]
        a = a.reshape(2 * NPAIRS, cfg.n_tiles, 4, 128)  # [k, t, c, p]
        a = a.reshape(NPAIRS, 2, cfg.n_tiles, 4, 128).transpose(2, 4, 0, 3, 1)
        idxh = np.ascontiguousarray(
            a.reshape(cfg.n_tiles, 128, IDX_PER_TILE)
        )
        centh = np.zeros((cfg.spad, CIN), dtype=BF16)
        centh[:spc] = feats[c * spc : (c + 1) * spc].astype(BF16)
        in_maps.append(
            {"tbl": tblh, "cent": centh, "idxs": idxh, "wts": wtsh,
             "bias": biash}
        )
    return in_maps


_CACHE = {}


def kernel(feats, kernel, bias, neighbor_map):
    feats = np.asarray(feats, dtype=np.float32)
    kern = np.asarray(kernel, dtype=np.float32)
    bias = np.asarray(bias, dtype=np.float32)

    n_tiles = (SPC + TILE - 1) // TILE  # 98
    cfg = Cfg(n_rows=N + 128, n_tiles=n_tiles)

    if "nc" not in _CACHE:
        _CACHE["nc"] = build(cfg)
    nc = _CACHE["nc"]

    in_maps = prep_inputs(feats, kern, bias, neighbor_map, cfg, N, NCORES)
    res = run_bass_kernel_spmd(nc, in_maps, list(range(NCORES)))
    out = np.concatenate(
        [res.results[i]["out"][:SPC] for i in range(NCORES)], axis=0
    )
    return out.astype(np.float32)


if __name__ == "__main__":
    # smoke test with random data
    rng = np.random.default_rng(0)
    feats = rng.standard_normal((N, CIN), dtype=np.float32)
    kern = rng.standard_normal((KVOL, CIN, COUT), dtype=np.float32) * 0.02
    bias = rng.standard_normal(COUT).astype(np.float32) * 0.02
    nm = rng.integers(0, N, (KVOL, N))
    out = kernel(feats, kern, bias, nm)
    print(out.shape, out.dtype)



# revision 2
# speedup vs baseline: 7.0234x; 7.0234x over previous
"""Sparse 3D conv (rulebook gather -> GEMM -> accumulate) on 8 TRN2 NeuronCores.

Strategy (data-parallel over output sites, no collectives):
  - Replicate the feats table (bf16) and the [27,64,64] kernel on every core.
  - Shard the 400k output sites 50k/core; each core gathers its neighbor
    rows via SWDGE indirect DMA (one 128-descriptor instruction per
    128-site column per kernel offset), transposes gathered [site, cin]
    tiles to [cin, site] on the TensorEngine (pairs of offsets packed to
    K=128), and accumulates 14 K=128-packed bf16 matmuls into f32 PSUM.
  - Bias add fused into the PSUM->SBUF copy, output transposed back to
    site-major on the TensorEngine and DMAd out contiguously.

Gather pipeline (v2):
  - num_swdge_queues=4 and gathers round-robined across SWDGE queues so one
    queue's SDMA drain overlaps the next one's descriptor generation.
  - Invalid rulebook entries (-1) and tail-pad sites point out-of-bounds and
    are SKIPPED at descriptor-generation time (bounds_check +
    oob_is_err=False) instead of fetching a zero row; g tiles are pre-zeroed
    on the Vector engine so skipped cells contribute exactly 0.
  - The 4 always-pad idx columns (offset 27 of the 14 offset pairs) are not
    gathered at all.
  - The center offset (k=13, always the site itself) is fed by plain HWDGE
    DMA instead of Pool-engine indirect gathers.

Measured on TRN2 (rep-delta, floor-free): 14.7 ms vs 16.0 ms for the
previous pipeline; the Pool engine is saturated by the irreducible
~1.43 us/instruction SWDGE cost at the HW cap of 128 descriptors
(one per partition) per indirect-DMA instruction.
"""

import sys
from contextlib import ExitStack

sys.path.insert(0, "/opt/trn_rl_repo")

import ml_dtypes
import numpy as np

import concourse.bass as bass
import concourse.tile as tile
from concourse import bacc, mybir
from concourse.bass_utils import run_bass_kernel_spmd
from concourse.masks import make_identity

BF16 = ml_dtypes.bfloat16

# Problem constants (hardcoded per contract)
N = 400000
CIN = 64
COUT = 64
KVOL = 27
NCORES = 8
SPC = N // NCORES  # sites per core = 50000

TILE = 512  # sites per device tile
NPAIRS = (KVOL + 1) // 2  # 14 (27 offsets + 1 zero pad)
IDX_PER_TILE = NPAIRS * 8  # 112 idx columns per tile

CQ = (49, 51, 53, 55)  # center offset (k=13) columns: direct HWDGE DMA
PQ = (105, 107, 109, 111)  # pad offset (k=27) columns: never gathered

OOB_IDX = np.int32(N + 64)  # > bounds_check=N-1 -> descriptor skipped


class Cfg:
    def __init__(self, n_rows, n_tiles):
        self.n_rows = n_rows  # feats table rows incl. zero pad rows
        self.n_tiles = n_tiles  # site tiles per core
        self.spad = n_tiles * TILE  # padded sites per core


def build(cfg: Cfg, skip=True, nq=4, reps=1):
    """Build + compile the per-core Bass program."""
    nc = bacc.Bacc(
        "TRN2",
        target_bir_lowering=False,
        debug=False,
        num_devices=NCORES,
        num_swdge_queues=nq,
    )
    f32 = mybir.dt.float32
    bf16 = mybir.dt.bfloat16
    i32 = mybir.dt.int32

    tbl = nc.dram_tensor("tbl", [cfg.n_rows, CIN], bf16, kind="ExternalInput")
    cent = nc.dram_tensor("cent", [cfg.spad, CIN], bf16, kind="ExternalInput")
    idxs = nc.dram_tensor(
        "idxs", [cfg.n_tiles, 128, IDX_PER_TILE], i32, kind="ExternalInput"
    )
    wts = nc.dram_tensor("wts", [128, NPAIRS * COUT], bf16, kind="ExternalInput")
    biasd = nc.dram_tensor("bias", [COUT, 1], f32, kind="ExternalInput")
    outd = nc.dram_tensor("out", [cfg.spad, COUT], f32, kind="ExternalOutput")

    with tile.TileContext(nc) as tc, ExitStack() as ctx:
        const = ctx.enter_context(tc.tile_pool(name="const", bufs=1))
        idf = const.tile([128, 128], f32)
        make_identity(nc, idf[:])
        idb = const.tile([128, 128], bf16)
        nc.vector.tensor_copy(idb[:], idf[:])
        wt = const.tile([128, NPAIRS * COUT], bf16)
        nc.sync.dma_start(wt[:], wts[:])
        bt = const.tile([COUT, 1], f32)
        nc.sync.dma_start(bt[:], biasd[:])

        ip = ctx.enter_context(tc.tile_pool(name="ip", bufs=3))
        gp = ctx.enter_context(tc.tile_pool(name="gp", bufs=3))
        tp = ctx.enter_context(tc.tile_pool(name="tp", bufs=2, space="PSUM"))
        rp = ctx.enter_context(tc.tile_pool(name="rp", bufs=6))
        app = ctx.enter_context(tc.tile_pool(name="ap", bufs=2, space="PSUM"))
        bp = ctx.enter_context(tc.tile_pool(name="bp", bufs=2))
        otp = ctx.enter_context(tc.tile_pool(name="otp", bufs=2, space="PSUM"))
        osp = ctx.enter_context(tc.tile_pool(name="osp", bufs=3))

        gq = [0]

        def gather(out_ap, idx_ap):
            kw = {}
            if skip:
                kw = dict(bounds_check=N - 1, oob_is_err=False)
            inst = nc.gpsimd.indirect_dma_start(
                out=out_ap,
                out_offset=None,
                in_=tbl[:],
                in_offset=bass.IndirectOffsetOnAxis(ap=idx_ap, axis=0),
                **kw,
            )
            if nq > 1:
                q = gq[0] % nq
                gq[0] += 1
                if q:
                    inst.ins.queue = f"qPoolDynamic{q}"
            return inst

        for _rep in range(reps):
            for t in range(cfg.n_tiles):
                it = ip.tile([128, IDX_PER_TILE], i32)
                nc.sync.dma_start(it[:], idxs[t])
                acc = app.tile([COUT, TILE], f32)
                g = gp.tile([128, IDX_PER_TILE * CIN], bf16)
                if skip:
                    nc.vector.memset(g[:], 0.0)
                for gi in range(IDX_PER_TILE):
                    if gi in CQ:
                        continue
                    if skip and gi in PQ:
                        continue
                    gather(
                        g[:, gi * CIN : (gi + 1) * CIN],
                        it[:, gi : gi + 1],
                    )
                for c in range(4):
                    q = CQ[c]
                    nc.sync.dma_start(
                        g[:, q * CIN : (q + 1) * CIN],
                        cent[t * TILE + c * 128 : t * TILE + (c + 1) * 128],
                    )
                for pr in range(NPAIRS):
                    tpt = tp.tile([128, TILE], bf16)
                    for c in range(4):
                        nc.tensor.transpose(
                            out=tpt[:, c * 128 : (c + 1) * 128],
                            in_=g[
                                :, pr * 512 + c * 128 : pr * 512 + (c + 1) * 128
                            ],
                            identity=idb[:],
                        )
                    r = rp.tile([128, TILE], bf16)
                    if pr % 2 == 0:
                        nc.vector.tensor_copy(r[:], tpt[:])
                    else:
                        nc.scalar.copy(r[:], tpt[:])
                    nc.tensor.matmul(
                        acc[:],
                        wt[:, pr * COUT : (pr + 1) * COUT],
                        r[:],
                        start=(pr == 0),
                        stop=(pr == NPAIRS - 1),
                    )
                ob = bp.tile([COUT, TILE], f32)
                nc.vector.tensor_add(
                    out=ob[:], in0=acc[:], in1=bt[:].to_broadcast([COUT, TILE])
                )
                ot = otp.tile([128, 4 * COUT], f32)
                for c in range(4):
                    nc.tensor.transpose(
                        out=ot[:, c * COUT : (c + 1) * COUT],
                        in_=ob[:, c * 128 : (c + 1) * 128],
                        identity=idf[:COUT, :COUT],
                    )
                os_ = osp.tile([128, 4 * COUT], f32)
                nc.scalar.copy(os_[:], ot[:])
                nc.sync.dma_start(
                    outd[t * TILE : (t + 1) * TILE, :].rearrange(
                        "(c p) ci -> p c ci", p=128
                    ),
                    os_[:].rearrange("p (c ci) -> p c ci", c=4),
                )

    nc.compile()
    return nc


def prep_inputs(feats, kern, bias, neighbor_map, cfg: Cfg, skip=True):
    """Host-side marshalling into per-core input maps."""
    zrow = N  # zero row: belt-and-suspenders target if OOB skip ever regresses
    tblh = np.zeros((cfg.n_rows, CIN), dtype=BF16)
    tblh[:N] = feats.astype(BF16)

    nm = np.asarray(neighbor_map)
    inval = OOB_IDX if skip else np.int32(zrow)
    idx32 = np.where(nm >= 0, nm, inval).astype(np.int32)  # [KVOL, N]

    w_pk = np.zeros((NPAIRS, 128, COUT), dtype=np.float32)
    for pr in range(NPAIRS):
        k0, k1 = 2 * pr, 2 * pr + 1
        w_pk[pr, :CIN] = kern[k0]
        if k1 < KVOL:
            w_pk[pr, CIN:] = kern[k1]
    wtsh = np.ascontiguousarray(
        w_pk.transpose(1, 0, 2).reshape(128, NPAIRS * COUT)
    ).astype(BF16)

    biash = np.ascontiguousarray(bias.reshape(COUT, 1)).astype(np.float32)

    in_maps = []
    for c in range(NCORES):
        sl = idx32[:, c * SPC : (c + 1) * SPC]  # [27, SPC]
        padn = cfg.spad - SPC
        a = np.concatenate(
            [
                np.concatenate(
                    [sl, np.full((KVOL, padn), inval, np.int32)], axis=1
                ),
                np.full((1, cfg.spad), inval, np.int32),
            ],
            axis=0,
        )  # [28, spad]
        a = a.reshape(2 * NPAIRS, cfg.n_tiles, 4, 128)  # [k, t, c, p]
        a = a.reshape(NPAIRS, 2, cfg.n_tiles, 4, 128).transpose(2, 4, 0, 3, 1)
        idxh = np.ascontiguousarray(a.reshape(cfg.n_tiles, 128, IDX_PER_TILE))
        centh = np.zeros((cfg.spad, CIN), dtype=BF16)
        centh[:SPC] = feats[c * SPC : (c + 1) * SPC].astype(BF16)
        in_maps.append(
            {"tbl": tblh, "cent": centh, "idxs": idxh, "wts": wtsh,
             "bias": biash}
        )
    return in_maps


_CACHE = {}


def kernel(feats, kernel, bias, neighbor_map):
    feats = np.asarray(feats, dtype=np.float32)
    kern = np.asarray(kernel, dtype=np.float32)
    bias = np.asarray(bias, dtype=np.float32)

    n_tiles = (SPC + TILE - 1) // TILE  # 98
    cfg = Cfg(n_rows=N + 128, n_tiles=n_tiles)

    if "nc" not in _CACHE:
        _CACHE["nc"] = build(cfg)
    nc = _CACHE["nc"]

    in_maps = prep_inputs(feats, kern, bias, neighbor_map, cfg)
    res = run_bass_kernel_spmd(nc, in_maps, list(range(NCORES)))
    out = np.concatenate(
        [res.results[i]["out"][:SPC] for i in range(NCORES)], axis=0
    )
    return out.astype(np.float32)


if __name__ == "__main__":
    # smoke test with random data
    rng = np.random.default_rng(0)
    feats = rng.standard_normal((N, CIN), dtype=np.float32)
    kern = rng.standard_normal((KVOL, CIN, COUT), dtype=np.float32) * 0.02
    bias = rng.standard_normal(COUT).astype(np.float32) * 0.02
    nm = rng.integers(-1, N, (KVOL, N))
    out = kernel(feats, kern, bias, nm)
    print(out.shape, out.dtype)
